# revision 25
# baseline (speedup 1.0000x reference)
"""Trainium2 Bass kernel for nn_CBSA_45389214384209 (sparse_attention).

Reference computation (per batch element b of 8):
  x_seq = x[b].T                      # [4096, 256]   (x[b] is [256, 4096])
  proj  = x_seq @ W_proj              # [4096, 512]
  rep   = avgpool8x8(proj)            # [64, 512]
  per head h (8 heads, dh=64):
    S    = rep_h @ proj_h.T * scale   # [64, 4096]
    P    = softmax(S)                 # [64, 4096]
    rd   = P @ proj_h                 # [64, 64]
    rep2 = rep_h + step_rep[h] * rd
    P2   = softmax(rep2 @ rep2.T * scale)
    xd2  = step_x[h] * (P2 @ rep2)    # [64, 64]
    xdT  = xd2.T @ P                  # [64, 4096]  (back-projection)
  out[b] = W_out.T @ concat_h(xdT) + b_out[:, None]   # [256, 4096]

Kernel structure (heads packed in pairs into 128-wide tiles throughout):
  * x lands via two parallel HWDGE rings (sync + scalar) in 1MB pieces and
    is consumed in place through f32r bitcast views -- no staging casts.
  * Pooling commutes with the projection: rep^T = Wp^T pool8x8(x) / 64.
  * Scores come straight from x via fused weights: S_p = (Wp repT_bd_p)^T x
    with repT_bd the SCALE-scaled block-diagonal pooled queries -- the
    transposed projection is never materialized.
  * Phase 3 runs group-major (pair p, quarter g): 4 score matmuls -> one
    1024-wide exp (PSUM 2-bank read) -> one DMA-xbar transpose (rings
    alternate) -> an 8-matmul rep_delta burst two slots later.  The PE
    stream therefore never waits on ACT or the transpose stream.
  * Stage 2 uses exp(S2) symmetry: S2^T is computed with swapped matmul
    operands and exp-ed directly into the [64,128] layout (no P2 transpose,
    no P2 normalization -- 1/Z2 folds into the V row scale), and xd2^T is
    produced directly by swapping the xd2 matmul operands.
  * Back-projection + output projection are fused algebraically:
    out = sum_h (Wo_h^T xd2_h^T) @ P_h + b_out, via per-pair V^T =
    xd2_bd^T @ Wo_pair -- a single K=512 accumulation over 4 pairs.

Sharding: pure data parallel - one batch element per NeuronCore (8 cores).
"""

import os
import sys

import numpy as np

for _p in ("/opt/trn_rl_repo", os.path.expanduser("~/.axon_site/_ro/trn_rl_repo")):
    if os.path.isdir(_p) and _p not in sys.path:
        sys.path.insert(0, _p)

import concourse.bass as bass
import concourse.tile as tile
from concourse import bacc, mybir
from concourse.bass import ds, ts
from concourse.masks import make_identity

F32 = mybir.dt.float32
FP8 = mybir.dt.float8e4
F32R = mybir.dt.float32r
BF16 = mybir.dt.bfloat16
AX = mybir.AxisListType
ALU = mybir.AluOpType
ACTF = mybir.ActivationFunctionType

B = 8
C = 256          # model dim
T = 4096         # tokens (64x64 grid)
INNER = 512
HEADS = 8
DH = 64
NB = 64          # pooled tokens (8x8 grid)
SCALE = DH ** -0.5
NPAIR = 4        # head pairs
NTT = 32         # 128-wide token tiles
NPIECE = 4       # 1024-wide x ingest pieces
NG = 4           # 1024-wide score/exp groups per pair

CFG = {"p_mode": "bf16"}


def build_module(cfg=CFG):
    nc = bacc.Bacc("TRN2", debug=False)

    x = nc.dram_tensor("x", [C, T], F32, kind="ExternalInput").ap()
    wp = nc.dram_tensor("w_proj", [C, INNER], F32, kind="ExternalInput").ap()
    wo = nc.dram_tensor("w_out", [INNER, C], F32, kind="ExternalInput").ap()
    bo = nc.dram_tensor("b_out", [C], F32, kind="ExternalInput").ap()
    srep = nc.dram_tensor("s_rep", [HEADS], F32, kind="ExternalInput").ap()
    sx = nc.dram_tensor("s_x", [HEADS], F32, kind="ExternalInput").ap()
    out = nc.dram_tensor("out", [C, T], F32, kind="ExternalOutput").ap()

    with tile.TileContext(nc) as tc:
        _body(tc, cfg, x, wp, wo, bo, srep, sx, out)
    nc.compile()
    return nc


def _body(tc, cfg, x, wp, wo, bo, srep, sx, out):
    nc = tc.nc

    x_r = x.rearrange("(o p) t -> p o t", p=128)      # [128, 2, 4096]
    out_r = out.rearrange("(o p) t -> p o t", p=128)  # [128, 2, 4096]
    wp_v = wp.rearrange("(o p) i -> p o i", p=128)    # [128, 2, 512]

    # ---- pools ------------------------------------------------------------
    consts = tc.alloc_tile_pool(name="consts", bufs=1)
    stats = tc.alloc_tile_pool(name="stats", bufs=1)
    vp = tc.alloc_tile_pool(name="vp", bufs=1)           # V^T per pair
    pp = tc.alloc_tile_pool(name="pp", bufs=1)           # P (attn) tiles
    b3 = tc.alloc_tile_pool(name="b3", bufs=2)           # stage-2 temps
    ptp = tc.alloc_tile_pool(name="ptp", bufs=1)         # P^T
    pnp = tc.alloc_tile_pool(name="pnp", bufs=1)         # proj (t-partition)
    xp = tc.alloc_tile_pool(name="xp", bufs=1)           # x (f32, in place)
    outp = tc.alloc_tile_pool(name="outp", bufs=1)       # out staging

    # PSUM budget is 8 banks: sm (2) + acc (2) + smm (2 x 2 banks).  The
    # 512-wide proj and output-stage accumulators rotate through the smm
    # tag's two 2-bank slots (using their first bank only) -- the three
    # phases are temporally disjoint.
    psA = tc.alloc_tile_pool(name="psA", bufs=1, space="PSUM")   # sm/acc
    psS = tc.alloc_tile_pool(name="psS", bufs=1, space="PSUM")   # smm

    # ---- input DMAs -------------------------------------------------------
    # The two HWDGE rings share ~200 GB/s of HBM read bandwidth, so x goes
    # as 8 FIFO chunks on ONE ring (in-order arrival -> per-chunk compute
    # chases the stream); wo rides the same ring behind x (needed late).
    # Only the tiny late-phase constants use the scalar ring.
    wp_sb = consts.tile([128, 2, INNER], F32, name="wp_sb")
    nc.sync.dma_start(wp_sb, wp_v)
    x_sb = xp.tile([128, 2, T], F32, name="x_sb")
    for j in range(8):
        nc.sync.dma_start(x_sb[:, :, ts(j, 512)], x_r[:, :, ts(j, 512)])
    wo_sb = consts.tile([128, 4, C], F32, name="wo_sb")
    nc.sync.dma_start(wo_sb, wo.rearrange("(g p) c -> p g c", p=128))
    bo_sb = consts.tile([128, 2], F32, name="bo_sb")
    nc.scalar.dma_start(bo_sb, bo.rearrange("(o p) -> p o", p=128))
    srep_ld = consts.tile([128, HEADS], F32, name="srep_ld")
    sx_ld = consts.tile([128, HEADS], F32, name="sx_ld")
    for st_dram, st_ld in ((srep, srep_ld), (sx, sx_ld)):
        bcast = bass.AP(
            tensor=st_dram.tensor, offset=st_dram.offset,
            ap=[[0, 128], [st_dram.ap[0][0], HEADS]],
        )
        nc.scalar.dma_start(st_ld, bcast)

    # proj/scores run in bf16: bf16 weight loads are separate LDWEIGHTS
    # instructions the PE pulls ahead (f32r matmuls self-load serially).
    # x casts run per chunk, split ACT/DVE.
    x_bf = xp.tile([128, 2, T], BF16, name="x_bf")
    # fp8 copy of x for the DoubleRow score matmuls (K=256 in one pass);
    # the bf16->fp8 chunk casts ride the otherwise-idle GPSIMD.
    x8 = xp.tile([128, 2, T], FP8, name="x8")
    wp_bf = consts.tile([128, 2, INNER], BF16, name="wp_bf")
    nc.gpsimd.tensor_copy(wp_bf, wp_sb)
    wp_r = consts.tile([128, 2, INNER], F32R, name="wp_r")
    nc.vector.tensor_copy(wp_r, wp_sb)

    ident_bf = consts.tile([128, 128], BF16, name="ident_bf")
    make_identity(nc, ident_bf)
    ident_f = consts.tile([128, 128], F32, name="ident_f")
    make_identity(nc, ident_f)

    # Wp^T (for the fused score weights Ws = Wp @ repT_bd)
    wpT_r = consts.tile([128, 4, C], F32R, name="wpT_r")
    for k in range(4):
        for o in range(2):
            wt_ps = psA.tile([128, 128], F32, name="wt_ps", tag="sm", bufs=2)
            nc.tensor.transpose(wt_ps, wp_sb[:, o, ts(k, 128)], ident_f)
            nc.vector.tensor_copy(wpT_r[:, k, ds(128 * o, 128)], wt_ps)
    # Ws entries are ~0.01-scale: store 64*Ws in fp8 (1/64 folds into the
    # exp activation scale) to stay in e4m3 normal range.
    ws8 = consts.tile([128, 2, NPAIR, 128], FP8, name="ws8")

    # step_rep / step_x broadcast per pair: column p holds step[2p] on
    # partitions 0-63 and step[2p+1] on partitions 64-127.
    ones_row = consts.tile([1, 128], F32, name="ones_row")
    nc.vector.memset(ones_row, 1.0)
    srep_bc = consts.tile([128, NPAIR], F32, name="srep_bc")
    sx_bc = consts.tile([128, NPAIR], F32, name="sx_bc")
    for st_ld, st_bc in ((srep_ld, srep_bc), (sx_ld, sx_bc)):
        st_ldv = st_ld.rearrange("p (c two) -> p c two", two=2)
        for half in range(2):
            rows = slice(64 * half, 64 * half + 64)
            nc.gpsimd.tensor_copy(st_bc[rows, :], st_ldv[rows, :, half])
    wo_bf = consts.tile([128, 4, C], BF16, name="wo_bf")
    nc.gpsimd.tensor_copy(wo_bf, wo_sb)

    # ---- projection pass + pooling, per 1024-token piece ------------------
    # proj = x^T Wp in token-partition layout (rep_delta lhsT); 8x8 block
    # sums of x ride on DVE.  PSUM exports alternate ACT/DVE.
    xpool_raw = consts.tile([128, 2, NB], F32, name="xpool_raw")
    proj_bf = pnp.tile([128, NTT, INNER], BF16, name="proj_bf")

    # repT = Wp^T xpool / 64; repT_bd[:, p, :] is [[sA, 0], [0, sB]]
    # (exact zeros kill the cross-head terms).  The scores then come
    # straight from x: S_p = repT_bd_p^T (Wp^T x) = (Wp repT_bd_p)^T x.
    # Issued between the last two proj blocks (its xpool inputs are ready
    # by then) so the scores can start right behind the projection tail.
    xpool_r = consts.tile([128, 2, NB], F32R, name="xpool_r")
    repT = consts.tile([128, NPAIR, NB], F32, name="repT")
    repT_bd = consts.tile([128, NPAIR, 128], F32, name="repT_bd")
    repT_bd_r = consts.tile([128, NPAIR, 128], F32R, name="repT_bd_r")

    def _build_ws():
        nc.vector.tensor_scalar_mul(xpool_r, xpool_raw, 1.0 / 64.0)
        nc.vector.memset(repT_bd, 0.0)
        for g in range(4):
            rt_ps = psA.tile([128, NB], F32, name="rt_ps", tag="sm", bufs=2)
            for o in range(2):
                nc.tensor.matmul(
                    rt_ps, wp_r[:, o, ts(g, 128)], xpool_r[:, o, :],
                    start=(o == 0), stop=(o == 1),
                )
            nc.vector.tensor_copy(repT[:, g, :], rt_ps)
            for h in range(2):
                rows = slice(64 * h, 64 * h + 64)
                nc.vector.tensor_scalar_mul(
                    repT_bd[rows, g, ds(64 * h, 64)], repT[rows, g, :], SCALE
                )
        nc.vector.tensor_copy(repT_bd_r, repT_bd)
        for p in range(NPAIR):
            for o in range(2):
                ws_ps = psA.tile([128, 128], F32, name="ws_ps", tag="sm",
                                 bufs=2)
                nc.tensor.matmul(
                    ws_ps, wpT_r[:, p, ds(128 * o, 128)], repT_bd_r[:, p, :],
                    start=True, stop=True,
                )
                nc.vector.tensor_scalar_mul(ws8[:, o, p, :], ws_ps, 64.0)

    for j in range(8):
        # f32 -> bf16 casts, one engine per half
        nc.scalar.copy(x_bf[:, 0, ts(j, 512)], x_sb[:, 0, ts(j, 512)])
        nc.vector.tensor_copy(x_bf[:, 1, ts(j, 512)], x_sb[:, 1, ts(j, 512)])
        nc.gpsimd.tensor_copy(x8[:, :, ts(j, 512)], x_bf[:, :, ts(j, 512)])
        # chunk j covers h rows 8j..8j+8 (one block row);
        # local t = hi*64 + wb*8 + wi -> reduce (hi, wi) per block col wb.
        # The pool -> ws chain gates the whole scores phase: high priority
        # so the scheduler runs each pool as its chunk lands.
        with tc.high_priority():
            nc.vector.reduce_sum(
                xpool_raw[:, :, ts(j, 8)],
                x_sb[:, :, ts(j, 512)].rearrange(
                    "p o (hi wb wi) -> p o wb hi wi", hi=8, wb=8),
                axis=AX.XY,
            )
        if j == 7:
            with tc.high_priority(offset=200):
                _build_ws()
        for m in range(4 * j, 4 * j + 4):
            pr_full = psS.tile([128, 2, 512], F32, name="s_ps", tag="smm",
                               bufs=2)
            pr_ps = pr_full[:, 0, :]
            for o in range(2):
                nc.tensor.matmul(
                    pr_ps, x_bf[:, o, ts(m, 128)], wp_bf[:, o, :],
                    start=(o == 0), stop=(o == 1),
                )
            if m % 4 == 3:
                nc.vector.tensor_copy(proj_bf[:, m, :], pr_ps)
            else:
                nc.scalar.copy(proj_bf[:, m, :], pr_ps)

    # ---- phase 3: scores -> exp -> P^T transpose -> rep_delta bursts ------
    # Group (p, g) covers tokens [1024g, 1024g+1024) for pair p.  P stays
    # unnormalized; 1/Z folds into rep_delta (rz_bc) and V (rz*step_x).
    # |s| <~ 2: no max-subtract.
    p_tiles = [
        pp.tile([128, T], BF16, name=f"p{p}", tag=f"p{p}") for p in range(NPAIR)
    ]
    pt_tiles = [
        ptp.tile([128, NTT, 128], BF16, name=f"pt{p}", tag=f"pt{p}")
        for p in range(NPAIR)
    ]
    zpart_tiles = [
        stats.tile([128, NG], F32, name="zpart", tag=f"zpart{p}")
        for p in range(NPAIR)
    ]

    def _scores(p, g):
        s_ps = psS.tile([128, 2, 512], F32, name="s_ps", tag="smm", bufs=2)
        for c2 in range(2):
            nc.tensor.matmul(
                s_ps[:, c2, :], ws8[:, :, p, :],
                x8[:, :, ds(1024 * g + 512 * c2, 512)],
                start=True, stop=True,
                perf_mode=mybir.MatmulPerfMode.DoubleRow,
            )
        nc.scalar.activation(
            out=p_tiles[p][:, ts(g, 1024)], in_=s_ps, func=ACTF.Exp,
            bias=0.0, scale=1.0 / 64.0, accum_out=zpart_tiles[p][:, g:g + 1],
        )
        eng = nc.sync if (NG * p + g) % 2 == 0 else nc.scalar
        eng.dma_start_transpose(
            pt_tiles[p][:, 8 * g:8 * g + 8, :], p_tiles[p][:, ts(g, 1024)]
        )

    av1_tiles = [None] * NPAIR

    def _av1_burst(p, g):
        if g == 0:
            av1_tiles[p] = psA.tile([128, 128], F32, name="av1_ps", tag="acc",
                                    bufs=2)
        av1_ps = av1_tiles[p]
        for m in range(8 * g, 8 * g + 8):
            nc.tensor.matmul(
                av1_ps, proj_bf[:, m, ds(128 * p, 128)], pt_tiles[p][:, m, :],
                start=(m == 0), stop=(m == NTT - 1),
            )

    # softmax normalizer chain for pair p (issue after its last exp)
    rz_tiles = [None] * NPAIR
    rzbc_tiles = [None] * NPAIR

    def _rz_chain(p):
        zsum = stats.tile([128, 1], F32, name="zsum", tag=f"zsum{p}")
        nc.vector.reduce_sum(zsum, zpart_tiles[p], axis=AX.X)
        rz = stats.tile([128, 1], F32, name="rz", tag=f"rz{p}")
        nc.vector.reciprocal(rz, zsum)
        rz_tiles[p] = rz
        rzt_ps = psA.tile([1, 128], F32, name="rzt_ps", tag="sm", bufs=2)
        nc.tensor.transpose(rzt_ps, rz, ident_f)
        rzt_sb = b3.tile([1, 128], F32, name="rzt_sb", tag="rzt_sb", bufs=4)
        nc.vector.tensor_copy(rzt_sb, rzt_ps)
        rzbc_ps = psA.tile([128, 128], F32, name="rzbc_ps", tag="sm", bufs=2)
        nc.tensor.matmul(rzbc_ps, ones_row, rzt_sb, start=True, stop=True)
        rz_bc = b3.tile([128, 128], F32, name="rz_bc", tag="rz_bc", bufs=2)
        nc.vector.tensor_copy(rz_bc, rzbc_ps)
        rzbc_tiles[p] = rz_bc

    # ---- stage 2 (per pair, issued behind its rep_delta) ------------------
    rep2Tb_tiles = [None] * NPAIR
    rep2Tbd_tiles = [None] * NPAIR
    r2_tiles = [None] * NPAIR
    p2_tiles = [None] * NPAIR
    rz2_tiles = [None] * NPAIR
    v_tiles = [None] * NPAIR

    def _build_rep2(p):
        # rep2^T = repT + step_rep * rz * rep_delta^T   [128 (dA|dB), 64 q]
        av1_ps, rz_bc = av1_tiles[p], rzbc_tiles[p]
        rep2T = b3.tile([128, NB], F32, name="rep2T", tag="rep2T", bufs=2)
        for h in range(2):
            pr = slice(64 * h, 64 * h + 64)
            blk = ds(64 * h, 64)
            nc.vector.scalar_tensor_tensor(
                rep2T[pr, :], av1_ps[pr, blk], srep_bc[pr, p:p + 1],
                rz_bc[pr, blk], op0=ALU.mult, op1=ALU.mult,
            )
        nc.vector.tensor_add(rep2T, rep2T, repT[:, p, :])
        rep2T_b = b3.tile([128, NB], BF16, name="rep2T_b", tag="rep2T_b", bufs=4)
        nc.vector.tensor_copy(rep2T_b, rep2T)
        rep2Tb_tiles[p] = rep2T_b
        rep2T_bd = b3.tile([128, 128], BF16, name="rep2T_bd", tag="rep2T_bd",
                           bufs=4)
        nc.vector.memset(rep2T_bd, 0.0)
        for h in range(2):
            rows = slice(64 * h, 64 * h + 64)
            nc.vector.tensor_scalar_mul(
                rep2T_bd[rows, ds(64 * h, 64)], rep2T[rows, :], SCALE
            )
        rep2Tbd_tiles[p] = rep2T_bd

    def _stage2a(p):
        # rep2 (q' on partitions): [64 q', 128 (dA|dB)]
        r2_ps = psA.tile([64, 128], BF16, name="r2_ps", tag="sm", bufs=2)
        nc.tensor.transpose(r2_ps, rep2Tb_tiles[p], ident_bf)
        r2_sb = b3.tile([64, 128], BF16, name="r2_sb", tag="r2_sb", bufs=4)
        nc.vector.tensor_copy(r2_sb, r2_ps)
        r2_tiles[p] = r2_sb
        # S2 = (scale*rep2) @ rep2.T per head -> [128 (qA|qB), 64 q'].  Only
        # its exp row-sum (Z2) is consumed; exp(S2) itself is taken from the
        # transposed product below (exp(S2) is symmetric per head block).
        s2_ps = psA.tile([128, NB], F32, name="s2_ps", tag="sm", bufs=2)
        nc.tensor.matmul(s2_ps, rep2Tbd_tiles[p], rep2Tb_tiles[p],
                         start=True, stop=True)
        z2 = stats.tile([128, 1], F32, name="z2", tag=f"z2{p}")
        p2_sb = b3.tile([128, NB], BF16, name="p2_sb", tag="p2_sb", bufs=4)
        nc.scalar.activation(
            out=p2_sb, in_=s2_ps, func=ACTF.Exp,
            bias=0.0, scale=1.0, accum_out=z2,
        )
        # S2^T = rep2 @ (scale*rep2)^T -> [64 q', 128 (qA|qB)]; its exp IS
        # P2^T unnormalized (1/Z2 is folded into the V row scale).
        s2t_ps = psA.tile([64, 128], F32, name="s2t_ps", tag="sm", bufs=2)
        nc.tensor.matmul(s2t_ps, rep2Tb_tiles[p], rep2Tbd_tiles[p],
                         start=True, stop=True)
        p2t_sb = b3.tile([64, 128], BF16, name="p2t_sb", tag="p2t_sb", bufs=4)
        nc.scalar.activation(
            out=p2t_sb, in_=s2t_ps, func=ACTF.Exp, bias=0.0, scale=1.0,
        )
        p2_tiles[p] = p2t_sb
        rz2 = stats.tile([128, 1], F32, name="rz2", tag=f"rz2{p}")
        nc.vector.reciprocal(rz2, z2)
        rz2_tiles[p] = rz2

    def _stage2b(p):
        # xd2^T directly: [128 (dA|dB), 128 (qA|qB)] = rep2^T @ P2unnorm^T;
        # diag quadrants real, cross quadrants garbage (zeroed below).
        xd2t_ps = psA.tile([128, 128], F32, name="xd2t_ps", tag="sm", bufs=2)
        nc.tensor.matmul(xd2t_ps, r2_tiles[p], p2_tiles[p],
                         start=True, stop=True)
        xd2bd = b3.tile([128, 128], BF16, name="xd2bd", tag="xd2bd", bufs=4)
        nc.vector.memset(xd2bd, 0.0)
        for h in range(2):
            rows = slice(64 * h, 64 * h + 64)
            nc.vector.tensor_copy(
                xd2bd[rows, ds(64 * h, 64)], xd2t_ps[rows, ds(64 * h, 64)]
            )
        # V_pair^T[q, c] = sum_d xd2_bd^T[d, q] Wo[128p+d, c], then scale
        # rows (queries) by rz * step_x * rz2 (stage-1 and stage-2 softmax
        # normalizers both fold in here).
        v_ps = psA.tile([128, C], F32, name="v_ps", tag="sm", bufs=2)
        nc.tensor.matmul(v_ps, xd2bd, wo_bf[:, p, :], start=True, stop=True)
        rzsx = stats.tile([128, 1], F32, name="rzsx", tag=f"rzsx{p}")
        nc.vector.tensor_mul(rzsx, rz_tiles[p], sx_bc[:, p:p + 1])
        nc.vector.tensor_mul(rzsx, rzsx, rz2_tiles[p])
        v_sb = vp.tile([128, C], BF16, name=f"v{p}", tag=f"v{p}")
        nc.vector.tensor_scalar_mul(v_sb, v_ps, rzsx)
        v_tiles[p] = v_sb

    # pipeline driver: av1 bursts trail the score groups by 2 slots so the
    # PE stream never head-of-line blocks on an in-flight exp/transpose;
    # each pair's rep2 build (DVE) gets a slot of head start before its
    # stage-2 PE ops enter the stream.
    sched = [(p, g) for p in range(NPAIR) for g in range(NG)]
    nsl = len(sched)
    rep2_q = []
    s2_q = []
    for i in range(nsl + 4):
        if i < nsl:
            p, g = sched[i]
            _scores(p, g)
        if s2_q:
            pq = s2_q.pop(0)
            _stage2a(pq)
            _stage2b(pq)
        if rep2_q:
            pq = rep2_q.pop(0)
            _build_rep2(pq)
            s2_q.append(pq)
        if 2 <= i < nsl + 2:
            p2, g2 = sched[i - 2]
            if g2 == NG - 1:
                _rz_chain(p2)
            _av1_burst(p2, g2)
            if g2 == NG - 1:
                rep2_q.append(p2)

    # ---- fused back-projection + output projection -----------------------
    # out[c, t] = sum_p (V_pair^T)^T @ P_pair + b_out; K accumulates both
    # heads of the pair (query index carries head identity on both sides).
    for j in range(8):
        for ct in range(2):
            op_full = psS.tile([128, 2, 512], F32, name="s_ps", tag="smm",
                               bufs=2)
            op_ps = op_full[:, 0, :]
            for p in range(NPAIR):
                nc.tensor.matmul(
                    op_ps, v_tiles[p][:, ts(ct, 128)],
                    p_tiles[p][:, ts(j, 512)],
                    start=(p == 0), stop=(p == NPAIR - 1),
                )
            out_sb = outp.tile([128, 512], F32, name="out_sb", tag="out_sb",
                               bufs=4)
            if (2 * j + ct) % 2 == 0:
                nc.vector.tensor_tensor(
                    out_sb, op_ps, bo_sb[:, ct:ct + 1].to_broadcast((128, 512)),
                    ALU.add,
                )
            else:
                nc.scalar.activation(
                    out=out_sb, in_=op_ps, func=ACTF.Identity,
                    bias=bo_sb[:, ct:ct + 1], scale=1.0,
                )
            eng = nc.sync if (2 * j + ct) % 2 == 0 else nc.scalar
            eng.dma_start(out_r[:, ct, ts(j, 512)], out_sb)

    psS.release()
    psA.release()
    outp.release()
    xp.release()
    pnp.release()
    ptp.release()
    b3.release()
    pp.release()
    vp.release()
    stats.release()
    consts.release()


_CACHE = {}


class _Runner:
    """Builds the Bass module once and keeps a single jitted shard_map
    executable alive, so repeat kernel() calls skip retracing/relowering."""

    def __init__(self):
        import jax
        from jax.sharding import Mesh, PartitionSpec
        from jax.experimental.shard_map import shard_map
        from concourse import bass2jax

        self.jax = jax
        nc = build_module()
        self.nc = nc
        bass2jax.install_neuronx_cc_hook()

        partition_name = (
            nc.partition_id_tensor.name if nc.partition_id_tensor else None
        )
        in_names, out_names, out_avals = [], [], []
        for alloc in nc.m.functions[0].allocations:
            if not isinstance(alloc, mybir.MemoryLocationSet):
                continue
            name = alloc.memorylocations[0].name
            if alloc.kind == "ExternalInput":
                if name != partition_name:
                    in_names.append(name)
            elif alloc.kind == "ExternalOutput":
                out_names.append(name)
                out_avals.append(
                    jax.core.ShapedArray(
                        tuple(alloc.tensor_shape), mybir.dt.np(alloc.dtype)
                    )
                )
        n_params = len(in_names)
        n_outs = len(out_avals)
        all_names = list(in_names) + list(out_names)
        if partition_name is not None:
            all_names.append(partition_name)
        self.in_names = in_names
        self.out_names = out_names
        self.out_avals = out_avals

        def _body(*args):
            operands = list(args)
            if partition_name is not None:
                operands.append(bass2jax.partition_id_tensor())
            outs = bass2jax._bass_exec_p.bind(
                *operands,
                out_avals=tuple(out_avals),
                in_names=tuple(all_names),
                out_names=tuple(out_names),
                lowering_input_output_aliases=(),
                sim_require_finite=True,
                sim_require_nnan=True,
                nc=nc,
            )
            return tuple(outs)

        self.body = _body
        devices = jax.devices()[:B]
        mesh = Mesh(np.asarray(devices), ("core",))
        donate = tuple(range(n_params, n_params + n_outs))
        self.sharded = jax.jit(
            shard_map(
                _body, mesh=mesh,
                in_specs=(PartitionSpec("core"),) * (n_params + n_outs),
                out_specs=(PartitionSpec("core"),) * n_outs,
                check_rep=False,
            ),
            donate_argnums=donate,
            keep_unused=True,
        )

    def run(self, in_maps):
        concat_in = [
            np.concatenate([m[name] for m in in_maps], axis=0)
            for name in self.in_names
        ]
        zeros = [
            np.zeros((B * a.shape[0], *a.shape[1:]), a.dtype) for a in self.out_avals
        ]
        out_arrs = self.sharded(*concat_in, *zeros)
        return [
            {
                name: np.asarray(out_arrs[i]).reshape(B, *self.out_avals[i].shape)[c]
                for i, name in enumerate(self.out_names)
            }
            for c in range(B)
        ]

    def bench(self, in_maps, reps=8, inner=72, base=8):
        """Time device-resident executions (no donation, operands staged once).

        Times jitted chains of `base` and `inner` back-to-back kernel
        executions; returns (per_exec_seconds, base_chain_seconds, results)
        with per_exec = (t_inner - t_base) / (inner - base), which amortizes
        away the per-dispatch round-trip of this axon-tunneled environment.
        """
        import time
        from jax.sharding import Mesh, PartitionSpec, NamedSharding
        from jax.experimental.shard_map import shard_map

        jax = self.jax
        devices = jax.devices()[:B]
        mesh = Mesh(np.asarray(devices), ("core",))
        sharding = NamedSharding(mesh, PartitionSpec("core"))
        n_ops = len(self.in_names) + len(self.out_avals)

        def chain(n):
            def f(*args):
                outs = []
                for _ in range(n):
                    outs.extend(self.body(*args))
                return tuple(outs)
            return f

        concat_in = [
            np.concatenate([m[name] for m in in_maps], axis=0)
            for name in self.in_names
        ]
        zeros = [
            np.zeros((B * a.shape[0], *a.shape[1:]), a.dtype) for a in self.out_avals
        ]
        staged = [jax.device_put(a, sharding) for a in concat_in + zeros]

        # The device is occasionally in a degraded mode where chained
        # executions serialize (~10x): retry the whole measurement with
        # freshly traced executables and keep the best estimate.
        per_exec = float("inf")
        tbase_best = float("inf")
        out1 = None
        for attempt in range(4):
            times = {}
            for n in (base, inner):
                jfn = jax.jit(
                    shard_map(
                        chain(n), mesh=mesh,
                        in_specs=(PartitionSpec("core"),) * n_ops,
                        out_specs=(PartitionSpec("core"),) * (n * len(self.out_avals)),
                        check_rep=False,
                    ),
                    keep_unused=True,
                )
                out = jfn(*staged)
                jax.block_until_ready(out)
                best = float("inf")
                for _ in range(reps):
                    t0 = time.perf_counter()
                    out = jfn(*staged)
                    jax.block_until_ready(out)
                    best = min(best, time.perf_counter() - t0)
                times[n] = best
                if n == base and out1 is None:
                    out1 = out
            est = (times[inner] - times[base]) / (inner - base)
            if est <= 0:
                est = times[inner] / inner  # noise floor: report upper bound
            per_exec = min(per_exec, est)
            tbase_best = min(tbase_best, times[base])
            if per_exec < 1.2e-4:
                break

        results = [
            {
                name: np.asarray(out1[i]).reshape(B, *self.out_avals[i].shape)[c]
                for i, name in enumerate(self.out_names)
            }
            for c in range(B)
        ]
        return per_exec, tbase_best, results


def _get_runner():
    key = CFG["p_mode"]
    if key not in _CACHE:
        _CACHE[key] = _Runner()
    return _CACHE[key]


def _make_in_maps(x, W_proj, step_rep, step_x, W_out, b_out):
    x = np.ascontiguousarray(np.asarray(x, dtype=np.float32))
    shared = {
        "w_proj": np.ascontiguousarray(np.asarray(W_proj, dtype=np.float32)),
        "w_out": np.ascontiguousarray(np.asarray(W_out, dtype=np.float32)),
        "b_out": np.ascontiguousarray(np.asarray(b_out, dtype=np.float32)),
        "s_rep": np.ascontiguousarray(
            np.asarray(step_rep, dtype=np.float32).reshape(HEADS)
        ),
        "s_x": np.ascontiguousarray(
            np.asarray(step_x, dtype=np.float32).reshape(HEADS)
        ),
    }
    return [
        {"x": np.ascontiguousarray(x[b].reshape(C, T)), **shared} for b in range(B)
    ]


def kernel(x, W_proj, step_rep, step_x, W_out, b_out):
    runner = _get_runner()
    results = runner.run(_make_in_maps(x, W_proj, step_rep, step_x, W_out, b_out))
    outs = [np.asarray(results[b]["out"]).reshape(C, 64, 64) for b in range(B)]
    return np.stack(outs, axis=0)


# revision 26
# speedup vs baseline: 1.0646x; 1.0646x over previous
"""Trainium2 Bass kernel for nn_CBSA_45389214384209 (sparse_attention).

Reference computation (per batch element b of 8):
  x_seq = x[b].T                      # [4096, 256]   (x[b] is [256, 4096])
  proj  = x_seq @ W_proj              # [4096, 512]
  rep   = avgpool8x8(proj)            # [64, 512]
  per head h (8 heads, dh=64):
    S    = rep_h @ proj_h.T * scale   # [64, 4096]
    P    = softmax(S)                 # [64, 4096]
    rd   = P @ proj_h                 # [64, 64]
    rep2 = rep_h + step_rep[h] * rd
    P2   = softmax(rep2 @ rep2.T * scale)
    xd2  = step_x[h] * (P2 @ rep2)    # [64, 64]
    xdT  = xd2.T @ P                  # [64, 4096]  (back-projection)
  out[b] = W_out.T @ concat_h(xdT) + b_out[:, None]   # [256, 4096]

Kernel structure (heads packed in pairs into 128-wide tiles throughout):
  * x lands via two parallel HWDGE rings (sync + scalar) in 1MB pieces and
    is consumed in place through f32r bitcast views -- no staging casts.
  * Pooling commutes with the projection: rep^T = Wp^T pool8x8(x) / 64.
  * Scores come straight from x via fused weights: S_p = (Wp repT_bd_p)^T x
    with repT_bd the SCALE-scaled block-diagonal pooled queries -- the
    transposed projection is never materialized.
  * Phase 3 runs group-major (pair p, quarter g): 4 score matmuls -> one
    1024-wide exp (PSUM 2-bank read) -> one DMA-xbar transpose (rings
    alternate) -> an 8-matmul rep_delta burst two slots later.  The PE
    stream therefore never waits on ACT or the transpose stream.
  * Stage 2 uses exp(S2) symmetry: S2^T is computed with swapped matmul
    operands and exp-ed directly into the [64,128] layout (no P2 transpose,
    no P2 normalization -- 1/Z2 folds into the V row scale), and xd2^T is
    produced directly by swapping the xd2 matmul operands.
  * Back-projection + output projection are fused algebraically:
    out = sum_h (Wo_h^T xd2_h^T) @ P_h + b_out, via per-pair V^T =
    xd2_bd^T @ Wo_pair -- a single K=512 accumulation over 4 pairs.

Sharding: pure data parallel - one batch element per NeuronCore (8 cores).
"""

import os
import sys

import numpy as np

for _p in ("/opt/trn_rl_repo", os.path.expanduser("~/.axon_site/_ro/trn_rl_repo")):
    if os.path.isdir(_p) and _p not in sys.path:
        sys.path.insert(0, _p)

import concourse.bass as bass
import concourse.tile as tile
from concourse import bacc, mybir
from concourse.bass import ds, ts
from concourse.masks import make_identity

F32 = mybir.dt.float32
FP8 = mybir.dt.float8e4
F32R = mybir.dt.float32r
BF16 = mybir.dt.bfloat16
AX = mybir.AxisListType
ALU = mybir.AluOpType
ACTF = mybir.ActivationFunctionType

B = 8
C = 256          # model dim
T = 4096         # tokens (64x64 grid)
INNER = 512
HEADS = 8
DH = 64
NB = 64          # pooled tokens (8x8 grid)
SCALE = DH ** -0.5
NPAIR = 4        # head pairs
NTT = 32         # 128-wide token tiles
NPIECE = 4       # 1024-wide x ingest pieces
NG = 4           # 1024-wide score/exp groups per pair

CFG = {"p_mode": "bf16"}


def build_module(cfg=CFG):
    nc = bacc.Bacc("TRN2", debug=False)

    x = nc.dram_tensor("x", [C, T], F32, kind="ExternalInput").ap()
    wp = nc.dram_tensor("w_proj", [C, INNER], F32, kind="ExternalInput").ap()
    wo = nc.dram_tensor("w_out", [INNER, C], F32, kind="ExternalInput").ap()
    bo = nc.dram_tensor("b_out", [C], F32, kind="ExternalInput").ap()
    srep = nc.dram_tensor("s_rep", [HEADS], F32, kind="ExternalInput").ap()
    sx = nc.dram_tensor("s_x", [HEADS], F32, kind="ExternalInput").ap()
    out = nc.dram_tensor("out", [C, T], F32, kind="ExternalOutput").ap()

    with tile.TileContext(nc) as tc:
        _body(tc, cfg, x, wp, wo, bo, srep, sx, out)
    nc.compile()
    return nc


def _body(tc, cfg, x, wp, wo, bo, srep, sx, out):
    nc = tc.nc

    x_r = x.rearrange("(o p) t -> p o t", p=128)      # [128, 2, 4096]
    out_r = out.rearrange("(o p) t -> p o t", p=128)  # [128, 2, 4096]
    wp_v = wp.rearrange("(o p) i -> p o i", p=128)    # [128, 2, 512]

    # ---- pools ------------------------------------------------------------
    consts = tc.alloc_tile_pool(name="consts", bufs=1)
    stats = tc.alloc_tile_pool(name="stats", bufs=1)
    vp = tc.alloc_tile_pool(name="vp", bufs=1)           # V^T per pair
    pp = tc.alloc_tile_pool(name="pp", bufs=1)           # P (attn) tiles
    b3 = tc.alloc_tile_pool(name="b3", bufs=2)           # stage-2 temps
    ptp = tc.alloc_tile_pool(name="ptp", bufs=1)         # P^T
    pnp = tc.alloc_tile_pool(name="pnp", bufs=1)         # proj (t-partition)
    xp = tc.alloc_tile_pool(name="xp", bufs=1)           # x (f32, in place)
    outp = tc.alloc_tile_pool(name="outp", bufs=1)       # out staging

    # PSUM budget is 8 banks: sm (2) + acc (2) + smm (2 x 2 banks).  The
    # 512-wide proj and output-stage accumulators rotate through the smm
    # tag's two 2-bank slots (using their first bank only) -- the three
    # phases are temporally disjoint.
    psA = tc.alloc_tile_pool(name="psA", bufs=1, space="PSUM")   # sm/acc
    psS = tc.alloc_tile_pool(name="psS", bufs=1, space="PSUM")   # smm

    # ---- input DMAs -------------------------------------------------------
    # The two HWDGE rings share ~200 GB/s of HBM read bandwidth, so x goes
    # as 8 FIFO chunks on ONE ring (in-order arrival -> per-chunk compute
    # chases the stream); wo rides the same ring behind x (needed late).
    # Only the tiny late-phase constants use the scalar ring.
    wp_sb = consts.tile([128, 2, INNER], F32, name="wp_sb")
    nc.sync.dma_start(wp_sb, wp_v)
    x_sb = xp.tile([128, 2, T], F32, name="x_sb")
    for j in range(8):
        nc.sync.dma_start(x_sb[:, :, ts(j, 512)], x_r[:, :, ts(j, 512)])
    wo_sb = consts.tile([128, 4, C], F32, name="wo_sb")
    nc.sync.dma_start(wo_sb, wo.rearrange("(g p) c -> p g c", p=128))
    bo_sb = consts.tile([128, 2], F32, name="bo_sb")
    nc.scalar.dma_start(bo_sb, bo.rearrange("(o p) -> p o", p=128))
    srep_ld = consts.tile([128, HEADS], F32, name="srep_ld")
    sx_ld = consts.tile([128, HEADS], F32, name="sx_ld")
    for st_dram, st_ld in ((srep, srep_ld), (sx, sx_ld)):
        bcast = bass.AP(
            tensor=st_dram.tensor, offset=st_dram.offset,
            ap=[[0, 128], [st_dram.ap[0][0], HEADS]],
        )
        nc.scalar.dma_start(st_ld, bcast)

    # proj/scores run in bf16: bf16 weight loads are separate LDWEIGHTS
    # instructions the PE pulls ahead (f32r matmuls self-load serially).
    # x casts run per chunk, split ACT/DVE.
    x_bf = xp.tile([128, 2, T], BF16, name="x_bf")
    # fp8 copy of x for the DoubleRow score matmuls (K=256 in one pass);
    # the bf16->fp8 chunk casts ride the otherwise-idle GPSIMD.
    x8 = xp.tile([128, 2, T], FP8, name="x8")
    wp_bf = consts.tile([128, 2, INNER], BF16, name="wp_bf")
    nc.gpsimd.tensor_copy(wp_bf, wp_sb)
    wp_r = consts.tile([128, 2, INNER], F32R, name="wp_r")
    nc.vector.tensor_copy(wp_r, wp_sb)

    ident_bf = consts.tile([128, 128], BF16, name="ident_bf")
    make_identity(nc, ident_bf)
    ident_f = consts.tile([128, 128], F32, name="ident_f")
    make_identity(nc, ident_f)

    # Wp^T (for the fused score weights Ws = Wp @ repT_bd)
    wpT_r = consts.tile([128, 4, C], F32R, name="wpT_r")
    for k in range(4):
        for o in range(2):
            wt_ps = psA.tile([128, 128], F32, name="wt_ps", tag="sm", bufs=2)
            nc.tensor.transpose(wt_ps, wp_sb[:, o, ts(k, 128)], ident_f)
            nc.vector.tensor_copy(wpT_r[:, k, ds(128 * o, 128)], wt_ps)
    # Ws entries are ~0.01-scale: store 64*Ws in fp8 (1/64 folds into the
    # exp activation scale) to stay in e4m3 normal range.
    ws8 = consts.tile([128, 2, NPAIR, 128], FP8, name="ws8")

    # step_rep / step_x broadcast per pair: column p holds step[2p] on
    # partitions 0-63 and step[2p+1] on partitions 64-127.
    ones_row = consts.tile([1, 128], F32, name="ones_row")
    nc.vector.memset(ones_row, 1.0)
    srep_bc = consts.tile([128, NPAIR], F32, name="srep_bc")
    sx_bc = consts.tile([128, NPAIR], F32, name="sx_bc")
    for st_ld, st_bc in ((srep_ld, srep_bc), (sx_ld, sx_bc)):
        st_ldv = st_ld.rearrange("p (c two) -> p c two", two=2)
        for half in range(2):
            rows = slice(64 * half, 64 * half + 64)
            nc.gpsimd.tensor_copy(st_bc[rows, :], st_ldv[rows, :, half])
    wo_bf = consts.tile([128, 4, C], BF16, name="wo_bf")
    nc.gpsimd.tensor_copy(wo_bf, wo_sb)

    # ---- projection pass + pooling, per 1024-token piece ------------------
    # proj = x^T Wp in token-partition layout (rep_delta lhsT); 8x8 block
    # sums of x ride on DVE.  PSUM exports alternate ACT/DVE.
    xpool_raw = consts.tile([128, 2, NB], F32, name="xpool_raw")
    proj_bf = pnp.tile([128, NTT, INNER], BF16, name="proj_bf")

    # repT = Wp^T xpool / 64; repT_bd[:, p, :] is [[sA, 0], [0, sB]]
    # (exact zeros kill the cross-head terms).  The scores then come
    # straight from x: S_p = repT_bd_p^T (Wp^T x) = (Wp repT_bd_p)^T x.
    # Issued between the last two proj blocks (its xpool inputs are ready
    # by then) so the scores can start right behind the projection tail.
    xpool_r = consts.tile([128, 2, NB], F32R, name="xpool_r")
    repT = consts.tile([128, NPAIR, NB], F32, name="repT")
    repT_bd = consts.tile([128, NPAIR, 128], F32, name="repT_bd")
    repT_bd_r = consts.tile([128, NPAIR, 128], F32R, name="repT_bd_r")

    def _build_ws():
        nc.vector.tensor_scalar_mul(xpool_r, xpool_raw, 1.0 / 64.0)
        nc.vector.memset(repT_bd, 0.0)
        for g in range(4):
            rt_ps = psA.tile([128, NB], F32, name="rt_ps", tag="sm", bufs=2)
            for o in range(2):
                nc.tensor.matmul(
                    rt_ps, wp_r[:, o, ts(g, 128)], xpool_r[:, o, :],
                    start=(o == 0), stop=(o == 1),
                )
            nc.vector.tensor_copy(repT[:, g, :], rt_ps)
            for h in range(2):
                rows = slice(64 * h, 64 * h + 64)
                nc.vector.tensor_scalar_mul(
                    repT_bd[rows, g, ds(64 * h, 64)], repT[rows, g, :], SCALE
                )
        nc.vector.tensor_copy(repT_bd_r, repT_bd)
        for p in range(NPAIR):
            for o in range(2):
                ws_ps = psA.tile([128, 128], F32, name="ws_ps", tag="sm",
                                 bufs=2)
                nc.tensor.matmul(
                    ws_ps, wpT_r[:, p, ds(128 * o, 128)], repT_bd_r[:, p, :],
                    start=True, stop=True,
                )
                nc.vector.tensor_scalar_mul(ws8[:, o, p, :], ws_ps, 64.0)

    for j in range(8):
        # f32 -> bf16 casts, one engine per half
        nc.scalar.copy(x_bf[:, 0, ts(j, 512)], x_sb[:, 0, ts(j, 512)])
        nc.vector.tensor_copy(x_bf[:, 1, ts(j, 512)], x_sb[:, 1, ts(j, 512)])
        # fp8 cast rotates engines: GPSIMD is slow (~3.5us/chunk) so it only
        # takes the last chunks, whose x8 is consumed latest.
        if j >= 6:
            nc.gpsimd.tensor_copy(x8[:, :, ts(j, 512)], x_sb[:, :, ts(j, 512)])
        elif j % 2 == 0:
            nc.vector.tensor_copy(x8[:, :, ts(j, 512)], x_sb[:, :, ts(j, 512)])
        else:
            nc.scalar.copy(x8[:, :, ts(j, 512)], x_sb[:, :, ts(j, 512)])
        # chunk j covers h rows 8j..8j+8 (one block row);
        # local t = hi*64 + wb*8 + wi -> reduce (hi, wi) per block col wb.
        # The pool -> ws chain gates the whole scores phase: high priority
        # so the scheduler runs each pool as its chunk lands.
        with tc.high_priority():
            nc.vector.reduce_sum(
                xpool_raw[:, :, ts(j, 8)],
                x_sb[:, :, ts(j, 512)].rearrange(
                    "p o (hi wb wi) -> p o wb hi wi", hi=8, wb=8),
                axis=AX.XY,
            )
        if j == 7:
            with tc.high_priority(offset=200):
                _build_ws()
        for m in range(4 * j, 4 * j + 4):
            pr_full = psS.tile([128, 2, 512], F32, name="s_ps", tag="smm",
                               bufs=2)
            pr_ps = pr_full[:, 0, :]
            for o in range(2):
                nc.tensor.matmul(
                    pr_ps, x_bf[:, o, ts(m, 128)], wp_bf[:, o, :],
                    start=(o == 0), stop=(o == 1),
                )
            if m % 4 == 3:
                nc.vector.tensor_copy(proj_bf[:, m, :], pr_ps)
            else:
                nc.scalar.copy(proj_bf[:, m, :], pr_ps)

    # ---- phase 3: scores -> exp -> P^T transpose -> rep_delta bursts ------
    # Group (p, g) covers tokens [1024g, 1024g+1024) for pair p.  P stays
    # unnormalized; 1/Z folds into rep_delta (rz_bc) and V (rz*step_x).
    # |s| <~ 2: no max-subtract.
    p_tiles = [
        pp.tile([128, T], BF16, name=f"p{p}", tag=f"p{p}") for p in range(NPAIR)
    ]
    pt_tiles = [
        ptp.tile([128, NTT, 128], BF16, name=f"pt{p}", tag=f"pt{p}")
        for p in range(NPAIR)
    ]
    zpart_tiles = [
        stats.tile([128, NG], F32, name="zpart", tag=f"zpart{p}")
        for p in range(NPAIR)
    ]

    def _scores(p, g):
        s_ps = psS.tile([128, 2, 512], F32, name="s_ps", tag="smm", bufs=2)
        for c2 in range(2):
            nc.tensor.matmul(
                s_ps[:, c2, :], ws8[:, :, p, :],
                x8[:, :, ds(1024 * g + 512 * c2, 512)],
                start=True, stop=True,
                perf_mode=mybir.MatmulPerfMode.DoubleRow,
            )
        nc.scalar.activation(
            out=p_tiles[p][:, ts(g, 1024)], in_=s_ps, func=ACTF.Exp,
            bias=0.0, scale=1.0 / 64.0, accum_out=zpart_tiles[p][:, g:g + 1],
        )
        eng = nc.sync if (NG * p + g) % 2 == 0 else nc.scalar
        eng.dma_start_transpose(
            pt_tiles[p][:, 8 * g:8 * g + 8, :], p_tiles[p][:, ts(g, 1024)]
        )

    av1_tiles = [None] * NPAIR

    def _av1_burst(p, g):
        if g == 0:
            av1_tiles[p] = psA.tile([128, 128], F32, name="av1_ps", tag="acc",
                                    bufs=2)
        av1_ps = av1_tiles[p]
        for m in range(8 * g, 8 * g + 8):
            nc.tensor.matmul(
                av1_ps, proj_bf[:, m, ds(128 * p, 128)], pt_tiles[p][:, m, :],
                start=(m == 0), stop=(m == NTT - 1),
            )

    # softmax normalizer chain for pair p (issue after its last exp)
    rz_tiles = [None] * NPAIR
    rzbc_tiles = [None] * NPAIR

    def _rz_chain(p):
        zsum = stats.tile([128, 1], F32, name="zsum", tag=f"zsum{p}")
        nc.vector.reduce_sum(zsum, zpart_tiles[p], axis=AX.X)
        rz = stats.tile([128, 1], F32, name="rz", tag=f"rz{p}")
        nc.vector.reciprocal(rz, zsum)
        rz_tiles[p] = rz
        rzt_ps = psA.tile([1, 128], F32, name="rzt_ps", tag="sm", bufs=2)
        nc.tensor.transpose(rzt_ps, rz, ident_f)
        rzt_sb = b3.tile([1, 128], F32, name="rzt_sb", tag="rzt_sb", bufs=4)
        nc.vector.tensor_copy(rzt_sb, rzt_ps)
        rzbc_ps = psA.tile([128, 128], F32, name="rzbc_ps", tag="sm", bufs=2)
        nc.tensor.matmul(rzbc_ps, ones_row, rzt_sb, start=True, stop=True)
        rz_bc = b3.tile([128, 128], F32, name="rz_bc", tag="rz_bc", bufs=2)
        nc.vector.tensor_copy(rz_bc, rzbc_ps)
        rzbc_tiles[p] = rz_bc

    # ---- stage 2 (per pair, issued behind its rep_delta) ------------------
    rep2Tb_tiles = [None] * NPAIR
    rep2Tbd_tiles = [None] * NPAIR
    r2_tiles = [None] * NPAIR
    p2_tiles = [None] * NPAIR
    rz2_tiles = [None] * NPAIR
    v_tiles = [None] * NPAIR

    def _build_rep2(p):
        # rep2^T = repT + step_rep * rz * rep_delta^T   [128 (dA|dB), 64 q]
        av1_ps, rz_bc = av1_tiles[p], rzbc_tiles[p]
        rep2T = b3.tile([128, NB], F32, name="rep2T", tag="rep2T", bufs=2)
        for h in range(2):
            pr = slice(64 * h, 64 * h + 64)
            blk = ds(64 * h, 64)
            nc.vector.scalar_tensor_tensor(
                rep2T[pr, :], av1_ps[pr, blk], srep_bc[pr, p:p + 1],
                rz_bc[pr, blk], op0=ALU.mult, op1=ALU.mult,
            )
        nc.vector.tensor_add(rep2T, rep2T, repT[:, p, :])
        rep2T_b = b3.tile([128, NB], BF16, name="rep2T_b", tag="rep2T_b", bufs=4)
        nc.vector.tensor_copy(rep2T_b, rep2T)
        rep2Tb_tiles[p] = rep2T_b
        rep2T_bd = b3.tile([128, 128], BF16, name="rep2T_bd", tag="rep2T_bd",
                           bufs=4)
        nc.vector.memset(rep2T_bd, 0.0)
        for h in range(2):
            rows = slice(64 * h, 64 * h + 64)
            nc.vector.tensor_scalar_mul(
                rep2T_bd[rows, ds(64 * h, 64)], rep2T[rows, :], SCALE
            )
        rep2Tbd_tiles[p] = rep2T_bd

    def _stage2a(p):
        # rep2 (q' on partitions): [64 q', 128 (dA|dB)]
        r2_ps = psA.tile([64, 128], BF16, name="r2_ps", tag="sm", bufs=2)
        nc.tensor.transpose(r2_ps, rep2Tb_tiles[p], ident_bf)
        r2_sb = b3.tile([64, 128], BF16, name="r2_sb", tag="r2_sb", bufs=4)
        nc.vector.tensor_copy(r2_sb, r2_ps)
        r2_tiles[p] = r2_sb
        # S2 = (scale*rep2) @ rep2.T per head -> [128 (qA|qB), 64 q'].  Only
        # its exp row-sum (Z2) is consumed; exp(S2) itself is taken from the
        # transposed product below (exp(S2) is symmetric per head block).
        s2_ps = psA.tile([128, NB], F32, name="s2_ps", tag="sm", bufs=2)
        nc.tensor.matmul(s2_ps, rep2Tbd_tiles[p], rep2Tb_tiles[p],
                         start=True, stop=True)
        z2 = stats.tile([128, 1], F32, name="z2", tag=f"z2{p}")
        p2_sb = b3.tile([128, NB], BF16, name="p2_sb", tag="p2_sb", bufs=4)
        nc.scalar.activation(
            out=p2_sb, in_=s2_ps, func=ACTF.Exp,
            bias=0.0, scale=1.0, accum_out=z2,
        )
        # S2^T = rep2 @ (scale*rep2)^T -> [64 q', 128 (qA|qB)]; its exp IS
        # P2^T unnormalized (1/Z2 is folded into the V row scale).
        s2t_ps = psA.tile([64, 128], F32, name="s2t_ps", tag="sm", bufs=2)
        nc.tensor.matmul(s2t_ps, rep2Tb_tiles[p], rep2Tbd_tiles[p],
                         start=True, stop=True)
        p2t_sb = b3.tile([64, 128], BF16, name="p2t_sb", tag="p2t_sb", bufs=4)
        nc.scalar.activation(
            out=p2t_sb, in_=s2t_ps, func=ACTF.Exp, bias=0.0, scale=1.0,
        )
        p2_tiles[p] = p2t_sb
        rz2 = stats.tile([128, 1], F32, name="rz2", tag=f"rz2{p}")
        nc.vector.reciprocal(rz2, z2)
        rz2_tiles[p] = rz2

    def _stage2b(p):
        # xd2^T directly: [128 (dA|dB), 128 (qA|qB)] = rep2^T @ P2unnorm^T;
        # diag quadrants real, cross quadrants garbage (zeroed below).
        xd2t_ps = psA.tile([128, 128], F32, name="xd2t_ps", tag="sm", bufs=2)
        nc.tensor.matmul(xd2t_ps, r2_tiles[p], p2_tiles[p],
                         start=True, stop=True)
        xd2bd = b3.tile([128, 128], BF16, name="xd2bd", tag="xd2bd", bufs=4)
        nc.vector.memset(xd2bd, 0.0)
        for h in range(2):
            rows = slice(64 * h, 64 * h + 64)
            nc.vector.tensor_copy(
                xd2bd[rows, ds(64 * h, 64)], xd2t_ps[rows, ds(64 * h, 64)]
            )
        # V_pair^T[q, c] = sum_d xd2_bd^T[d, q] Wo[128p+d, c], then scale
        # rows (queries) by rz * step_x * rz2 (stage-1 and stage-2 softmax
        # normalizers both fold in here).
        v_ps = psA.tile([128, C], F32, name="v_ps", tag="sm", bufs=2)
        nc.tensor.matmul(v_ps, xd2bd, wo_bf[:, p, :], start=True, stop=True)
        rzsx = stats.tile([128, 1], F32, name="rzsx", tag=f"rzsx{p}")
        nc.vector.tensor_mul(rzsx, rz_tiles[p], sx_bc[:, p:p + 1])
        nc.vector.tensor_mul(rzsx, rzsx, rz2_tiles[p])
        v_sb = vp.tile([128, C], BF16, name=f"v{p}", tag=f"v{p}")
        nc.vector.tensor_scalar_mul(v_sb, v_ps, rzsx)
        v_tiles[p] = v_sb

    # pipeline driver: av1 bursts trail the score groups by 2 slots so the
    # PE stream never head-of-line blocks on an in-flight exp/transpose;
    # each pair's rep2 build (DVE) gets a slot of head start before its
    # stage-2 PE ops enter the stream.
    sched = [(p, g) for p in range(NPAIR) for g in range(NG)]
    nsl = len(sched)
    rep2_q = []
    s2_q = []
    for i in range(nsl + 4):
        if i < nsl:
            p, g = sched[i]
            _scores(p, g)
        if s2_q:
            pq = s2_q.pop(0)
            _stage2a(pq)
            _stage2b(pq)
        if rep2_q:
            pq = rep2_q.pop(0)
            _build_rep2(pq)
            s2_q.append(pq)
        if 2 <= i < nsl + 2:
            p2, g2 = sched[i - 2]
            if g2 == NG - 1:
                _rz_chain(p2)
            _av1_burst(p2, g2)
            if g2 == NG - 1:
                rep2_q.append(p2)

    # ---- fused back-projection + output projection -----------------------
    # out[c, t] = sum_p (V_pair^T)^T @ P_pair + b_out; K accumulates both
    # heads of the pair (query index carries head identity on both sides).
    for j in range(8):
        for ct in range(2):
            op_full = psS.tile([128, 2, 512], F32, name="s_ps", tag="smm",
                               bufs=2)
            op_ps = op_full[:, 0, :]
            for p in range(NPAIR):
                nc.tensor.matmul(
                    op_ps, v_tiles[p][:, ts(ct, 128)],
                    p_tiles[p][:, ts(j, 512)],
                    start=(p == 0), stop=(p == NPAIR - 1),
                )
            out_sb = outp.tile([128, 512], F32, name="out_sb", tag="out_sb",
                               bufs=4)
            if (2 * j + ct) % 2 == 0:
                nc.vector.tensor_tensor(
                    out_sb, op_ps, bo_sb[:, ct:ct + 1].to_broadcast((128, 512)),
                    ALU.add,
                )
            else:
                nc.scalar.activation(
                    out=out_sb, in_=op_ps, func=ACTF.Identity,
                    bias=bo_sb[:, ct:ct + 1], scale=1.0,
                )
            eng = nc.sync if (2 * j + ct) % 2 == 0 else nc.scalar
            eng.dma_start(out_r[:, ct, ts(j, 512)], out_sb)

    psS.release()
    psA.release()
    outp.release()
    xp.release()
    pnp.release()
    ptp.release()
    b3.release()
    pp.release()
    vp.release()
    stats.release()
    consts.release()


_CACHE = {}


class _Runner:
    """Builds the Bass module once and keeps a single jitted shard_map
    executable alive, so repeat kernel() calls skip retracing/relowering."""

    def __init__(self):
        import jax
        from jax.sharding import Mesh, PartitionSpec
        from jax.experimental.shard_map import shard_map
        from concourse import bass2jax

        self.jax = jax
        nc = build_module()
        self.nc = nc
        bass2jax.install_neuronx_cc_hook()

        partition_name = (
            nc.partition_id_tensor.name if nc.partition_id_tensor else None
        )
        in_names, out_names, out_avals = [], [], []
        for alloc in nc.m.functions[0].allocations:
            if not isinstance(alloc, mybir.MemoryLocationSet):
                continue
            name = alloc.memorylocations[0].name
            if alloc.kind == "ExternalInput":
                if name != partition_name:
                    in_names.append(name)
            elif alloc.kind == "ExternalOutput":
                out_names.append(name)
                out_avals.append(
                    jax.core.ShapedArray(
                        tuple(alloc.tensor_shape), mybir.dt.np(alloc.dtype)
                    )
                )
        n_params = len(in_names)
        n_outs = len(out_avals)
        all_names = list(in_names) + list(out_names)
        if partition_name is not None:
            all_names.append(partition_name)
        self.in_names = in_names
        self.out_names = out_names
        self.out_avals = out_avals

        def _body(*args):
            operands = list(args)
            if partition_name is not None:
                operands.append(bass2jax.partition_id_tensor())
            outs = bass2jax._bass_exec_p.bind(
                *operands,
                out_avals=tuple(out_avals),
                in_names=tuple(all_names),
                out_names=tuple(out_names),
                lowering_input_output_aliases=(),
                sim_require_finite=True,
                sim_require_nnan=True,
                nc=nc,
            )
            return tuple(outs)

        self.body = _body
        devices = jax.devices()[:B]
        mesh = Mesh(np.asarray(devices), ("core",))
        donate = tuple(range(n_params, n_params + n_outs))
        self.sharded = jax.jit(
            shard_map(
                _body, mesh=mesh,
                in_specs=(PartitionSpec("core"),) * (n_params + n_outs),
                out_specs=(PartitionSpec("core"),) * n_outs,
                check_rep=False,
            ),
            donate_argnums=donate,
            keep_unused=True,
        )

    def run(self, in_maps):
        concat_in = [
            np.concatenate([m[name] for m in in_maps], axis=0)
            for name in self.in_names
        ]
        zeros = [
            np.zeros((B * a.shape[0], *a.shape[1:]), a.dtype) for a in self.out_avals
        ]
        out_arrs = self.sharded(*concat_in, *zeros)
        return [
            {
                name: np.asarray(out_arrs[i]).reshape(B, *self.out_avals[i].shape)[c]
                for i, name in enumerate(self.out_names)
            }
            for c in range(B)
        ]

    def bench(self, in_maps, reps=8, inner=72, base=8):
        """Time device-resident executions (no donation, operands staged once).

        Times jitted chains of `base` and `inner` back-to-back kernel
        executions; returns (per_exec_seconds, base_chain_seconds, results)
        with per_exec = (t_inner - t_base) / (inner - base), which amortizes
        away the per-dispatch round-trip of this axon-tunneled environment.
        """
        import time
        from jax.sharding import Mesh, PartitionSpec, NamedSharding
        from jax.experimental.shard_map import shard_map

        jax = self.jax
        devices = jax.devices()[:B]
        mesh = Mesh(np.asarray(devices), ("core",))
        sharding = NamedSharding(mesh, PartitionSpec("core"))
        n_ops = len(self.in_names) + len(self.out_avals)

        def chain(n):
            def f(*args):
                outs = []
                for _ in range(n):
                    outs.extend(self.body(*args))
                return tuple(outs)
            return f

        concat_in = [
            np.concatenate([m[name] for m in in_maps], axis=0)
            for name in self.in_names
        ]
        zeros = [
            np.zeros((B * a.shape[0], *a.shape[1:]), a.dtype) for a in self.out_avals
        ]
        staged = [jax.device_put(a, sharding) for a in concat_in + zeros]

        # The device is occasionally in a degraded mode where chained
        # executions serialize (~10x): retry the whole measurement with
        # freshly traced executables and keep the best estimate.
        per_exec = float("inf")
        tbase_best = float("inf")
        out1 = None
        for attempt in range(4):
            times = {}
            for n in (base, inner):
                jfn = jax.jit(
                    shard_map(
                        chain(n), mesh=mesh,
                        in_specs=(PartitionSpec("core"),) * n_ops,
                        out_specs=(PartitionSpec("core"),) * (n * len(self.out_avals)),
                        check_rep=False,
                    ),
                    keep_unused=True,
                )
                out = jfn(*staged)
                jax.block_until_ready(out)
                best = float("inf")
                for _ in range(reps):
                    t0 = time.perf_counter()
                    out = jfn(*staged)
                    jax.block_until_ready(out)
                    best = min(best, time.perf_counter() - t0)
                times[n] = best
                if n == base and out1 is None:
                    out1 = out
            est = (times[inner] - times[base]) / (inner - base)
            if est <= 0:
                est = times[inner] / inner  # noise floor: report upper bound
            per_exec = min(per_exec, est)
            tbase_best = min(tbase_best, times[base])
            if per_exec < 1.2e-4:
                break

        results = [
            {
                name: np.asarray(out1[i]).reshape(B, *self.out_avals[i].shape)[c]
                for i, name in enumerate(self.out_names)
            }
            for c in range(B)
        ]
        return per_exec, tbase_best, results


def _get_runner():
    key = CFG["p_mode"]
    if key not in _CACHE:
        _CACHE[key] = _Runner()
    return _CACHE[key]


def _make_in_maps(x, W_proj, step_rep, step_x, W_out, b_out):
    x = np.ascontiguousarray(np.asarray(x, dtype=np.float32))
    shared = {
        "w_proj": np.ascontiguousarray(np.asarray(W_proj, dtype=np.float32)),
        "w_out": np.ascontiguousarray(np.asarray(W_out, dtype=np.float32)),
        "b_out": np.ascontiguousarray(np.asarray(b_out, dtype=np.float32)),
        "s_rep": np.ascontiguousarray(
            np.asarray(step_rep, dtype=np.float32).reshape(HEADS)
        ),
        "s_x": np.ascontiguousarray(
            np.asarray(step_x, dtype=np.float32).reshape(HEADS)
        ),
    }
    return [
        {"x": np.ascontiguousarray(x[b].reshape(C, T)), **shared} for b in range(B)
    ]


def kernel(x, W_proj, step_rep, step_x, W_out, b_out):
    runner = _get_runner()
    results = runner.run(_make_in_maps(x, W_proj, step_rep, step_x, W_out, b_out))
    outs = [np.asarray(results[b]["out"]).reshape(C, 64, 64) for b in range(B)]
    return np.stack(outs, axis=0)


# revision 27
# speedup vs baseline: 1.0718x; 1.0068x over previous
"""Trainium2 Bass kernel for nn_CBSA_45389214384209 (sparse_attention).

Reference computation (per batch element b of 8):
  x_seq = x[b].T                      # [4096, 256]   (x[b] is [256, 4096])
  proj  = x_seq @ W_proj              # [4096, 512]
  rep   = avgpool8x8(proj)            # [64, 512]
  per head h (8 heads, dh=64):
    S    = rep_h @ proj_h.T * scale   # [64, 4096]
    P    = softmax(S)                 # [64, 4096]
    rd   = P @ proj_h                 # [64, 64]
    rep2 = rep_h + step_rep[h] * rd
    P2   = softmax(rep2 @ rep2.T * scale)
    xd2  = step_x[h] * (P2 @ rep2)    # [64, 64]
    xdT  = xd2.T @ P                  # [64, 4096]  (back-projection)
  out[b] = W_out.T @ concat_h(xdT) + b_out[:, None]   # [256, 4096]

Kernel structure (heads packed in pairs into 128-wide tiles throughout):
  * x lands via two parallel HWDGE rings (sync + scalar) in 1MB pieces and
    is consumed in place through f32r bitcast views -- no staging casts.
  * Pooling commutes with the projection: rep^T = Wp^T pool8x8(x) / 64.
  * Scores come straight from x via fused weights: S_p = (Wp repT_bd_p)^T x
    with repT_bd the SCALE-scaled block-diagonal pooled queries -- the
    transposed projection is never materialized.
  * Phase 3 runs group-major (pair p, quarter g): 4 score matmuls -> one
    1024-wide exp (PSUM 2-bank read) -> one DMA-xbar transpose (rings
    alternate) -> an 8-matmul rep_delta burst two slots later.  The PE
    stream therefore never waits on ACT or the transpose stream.
  * Stage 2 uses exp(S2) symmetry: S2^T is computed with swapped matmul
    operands and exp-ed directly into the [64,128] layout (no P2 transpose,
    no P2 normalization -- 1/Z2 folds into the V row scale), and xd2^T is
    produced directly by swapping the xd2 matmul operands.
  * Back-projection + output projection are fused algebraically:
    out = sum_h (Wo_h^T xd2_h^T) @ P_h + b_out, via per-pair V^T =
    xd2_bd^T @ Wo_pair -- a single K=512 accumulation over 4 pairs.

Sharding: pure data parallel - one batch element per NeuronCore (8 cores).
"""

import os
import sys

import numpy as np

for _p in ("/opt/trn_rl_repo", os.path.expanduser("~/.axon_site/_ro/trn_rl_repo")):
    if os.path.isdir(_p) and _p not in sys.path:
        sys.path.insert(0, _p)

import concourse.bass as bass
import concourse.tile as tile
from concourse import bacc, mybir
from concourse.bass import ds, ts
from concourse.masks import make_identity

F32 = mybir.dt.float32
FP8 = mybir.dt.float8e4
F32R = mybir.dt.float32r
BF16 = mybir.dt.bfloat16
AX = mybir.AxisListType
ALU = mybir.AluOpType
ACTF = mybir.ActivationFunctionType

B = 8
C = 256          # model dim
T = 4096         # tokens (64x64 grid)
INNER = 512
HEADS = 8
DH = 64
NB = 64          # pooled tokens (8x8 grid)
SCALE = DH ** -0.5
NPAIR = 4        # head pairs
NTT = 32         # 128-wide token tiles
NPIECE = 4       # 1024-wide x ingest pieces
NG = 4           # 1024-wide score/exp groups per pair

CFG = {"p_mode": "bf16"}


def build_module(cfg=CFG):
    nc = bacc.Bacc("TRN2", debug=False)

    x = nc.dram_tensor("x", [C, T], F32, kind="ExternalInput").ap()
    wp = nc.dram_tensor("w_proj", [C, INNER], F32, kind="ExternalInput").ap()
    wo = nc.dram_tensor("w_out", [INNER, C], F32, kind="ExternalInput").ap()
    bo = nc.dram_tensor("b_out", [C], F32, kind="ExternalInput").ap()
    srep = nc.dram_tensor("s_rep", [HEADS], F32, kind="ExternalInput").ap()
    sx = nc.dram_tensor("s_x", [HEADS], F32, kind="ExternalInput").ap()
    out = nc.dram_tensor("out", [C, T], F32, kind="ExternalOutput").ap()

    with tile.TileContext(nc) as tc:
        _body(tc, cfg, x, wp, wo, bo, srep, sx, out)
    nc.compile()
    return nc


def _body(tc, cfg, x, wp, wo, bo, srep, sx, out):
    nc = tc.nc

    x_r = x.rearrange("(o p) t -> p o t", p=128)      # [128, 2, 4096]
    out_r = out.rearrange("(o p) t -> p o t", p=128)  # [128, 2, 4096]
    wp_v = wp.rearrange("(o p) i -> p o i", p=128)    # [128, 2, 512]

    # ---- pools ------------------------------------------------------------
    consts = tc.alloc_tile_pool(name="consts", bufs=1)
    stats = tc.alloc_tile_pool(name="stats", bufs=1)
    vp = tc.alloc_tile_pool(name="vp", bufs=1)           # V^T per pair
    pp = tc.alloc_tile_pool(name="pp", bufs=1)           # P (attn) tiles
    b3 = tc.alloc_tile_pool(name="b3", bufs=2)           # stage-2 temps
    ptp = tc.alloc_tile_pool(name="ptp", bufs=1)         # P^T
    pnp = tc.alloc_tile_pool(name="pnp", bufs=1)         # proj (t-partition)
    xp = tc.alloc_tile_pool(name="xp", bufs=1)           # x + casts

    # PSUM budget is 8 banks: sm (2) + acc (2) + smm (2 x 2 banks).  The
    # 512-wide proj and output-stage accumulators rotate through the smm
    # tag's two 2-bank slots (using their first bank only) -- the three
    # phases are temporally disjoint.
    psA = tc.alloc_tile_pool(name="psA", bufs=1, space="PSUM")   # sm/acc
    psS = tc.alloc_tile_pool(name="psS", bufs=1, space="PSUM")   # smm

    # ---- input DMAs -------------------------------------------------------
    # The two HWDGE rings share ~200 GB/s of HBM read bandwidth, so x goes
    # as 8 FIFO chunks on ONE ring (in-order arrival -> per-chunk compute
    # chases the stream); wo rides the same ring behind x (needed late).
    # Only the tiny late-phase constants use the scalar ring.
    wp_sb = consts.tile([128, 2, INNER], F32, name="wp_sb")
    nc.sync.dma_start(wp_sb, wp_v)
    x_sb = xp.tile([128, 2, T], F32, name="x_sb")
    for j in range(8):
        nc.sync.dma_start(x_sb[:, :, ts(j, 512)], x_r[:, :, ts(j, 512)])
    wo_sb = consts.tile([128, 4, C], F32, name="wo_sb")
    nc.sync.dma_start(wo_sb, wo.rearrange("(g p) c -> p g c", p=128))
    bo_sb = consts.tile([128, 2], F32, name="bo_sb")
    nc.scalar.dma_start(bo_sb, bo.rearrange("(o p) -> p o", p=128))
    srep_ld = consts.tile([128, HEADS], F32, name="srep_ld")
    sx_ld = consts.tile([128, HEADS], F32, name="sx_ld")
    for st_dram, st_ld in ((srep, srep_ld), (sx, sx_ld)):
        bcast = bass.AP(
            tensor=st_dram.tensor, offset=st_dram.offset,
            ap=[[0, 128], [st_dram.ap[0][0], HEADS]],
        )
        nc.scalar.dma_start(st_ld, bcast)

    # proj/scores run in bf16: bf16 weight loads are separate LDWEIGHTS
    # instructions the PE pulls ahead (f32r matmuls self-load serially).
    # x casts run per chunk, split ACT/DVE.
    x_bf = xp.tile([128, 2, T], BF16, name="x_bf")
    # fp8 copy of x for the DoubleRow score matmuls (K=256 in one pass);
    # the bf16->fp8 chunk casts ride the otherwise-idle GPSIMD.
    x8 = xp.tile([128, 2, T], FP8, name="x8")
    wp_bf = consts.tile([128, 2, INNER], BF16, name="wp_bf")
    nc.gpsimd.tensor_copy(wp_bf, wp_sb)
    wp_r = consts.tile([128, 2, INNER], F32R, name="wp_r")
    nc.vector.tensor_copy(wp_r, wp_sb)

    ident_bf = consts.tile([128, 128], BF16, name="ident_bf")
    make_identity(nc, ident_bf)
    ident_f = consts.tile([128, 128], F32, name="ident_f")
    make_identity(nc, ident_f)

    # Wp^T (for the fused score weights Ws = Wp @ repT_bd)
    wpT_r = consts.tile([128, 4, C], F32R, name="wpT_r")
    for k in range(4):
        for o in range(2):
            wt_ps = psA.tile([128, 128], F32, name="wt_ps", tag="sm", bufs=2)
            nc.tensor.transpose(wt_ps, wp_sb[:, o, ts(k, 128)], ident_f)
            nc.vector.tensor_copy(wpT_r[:, k, ds(128 * o, 128)], wt_ps)
    # Ws entries are ~0.01-scale: store 64*Ws in fp8 (1/64 folds into the
    # exp activation scale) to stay in e4m3 normal range.
    ws8 = consts.tile([128, 2, NPAIR, 128], FP8, name="ws8")

    # step_rep / step_x broadcast per pair: column p holds step[2p] on
    # partitions 0-63 and step[2p+1] on partitions 64-127.
    ones_row = consts.tile([1, 128], F32, name="ones_row")
    nc.vector.memset(ones_row, 1.0)
    srep_bc = consts.tile([128, NPAIR], F32, name="srep_bc")
    sx_bc = consts.tile([128, NPAIR], F32, name="sx_bc")
    for st_ld, st_bc in ((srep_ld, srep_bc), (sx_ld, sx_bc)):
        st_ldv = st_ld.rearrange("p (c two) -> p c two", two=2)
        for half in range(2):
            rows = slice(64 * half, 64 * half + 64)
            nc.gpsimd.tensor_copy(st_bc[rows, :], st_ldv[rows, :, half])
    wo_bf = consts.tile([128, 4, C], BF16, name="wo_bf")
    nc.gpsimd.tensor_copy(wo_bf, wo_sb)

    # ---- projection pass + pooling, per 1024-token piece ------------------
    # proj = x^T Wp in token-partition layout (rep_delta lhsT); 8x8 block
    # sums of x ride on DVE.  PSUM exports alternate ACT/DVE.
    xpool_raw = consts.tile([128, 2, NB], F32, name="xpool_raw")
    proj_bf = pnp.tile([128, NTT, INNER], BF16, name="proj_bf")

    # repT = Wp^T xpool / 64; repT_bd[:, p, :] is [[sA, 0], [0, sB]]
    # (exact zeros kill the cross-head terms).  The scores then come
    # straight from x: S_p = repT_bd_p^T (Wp^T x) = (Wp repT_bd_p)^T x.
    # Issued between the last two proj blocks (its xpool inputs are ready
    # by then) so the scores can start right behind the projection tail.
    xpool_r = consts.tile([128, 2, NB], F32R, name="xpool_r")
    repT = consts.tile([128, NPAIR, NB], F32, name="repT")
    repT_bd = consts.tile([128, NPAIR, 128], F32, name="repT_bd")
    repT_bd_r = consts.tile([128, NPAIR, 128], F32R, name="repT_bd_r")

    def _build_ws():
        nc.vector.tensor_scalar_mul(xpool_r, xpool_raw, 1.0 / 64.0)
        nc.vector.memset(repT_bd, 0.0)
        for g in range(4):
            rt_ps = psA.tile([128, NB], F32, name="rt_ps", tag="sm", bufs=2)
            for o in range(2):
                nc.tensor.matmul(
                    rt_ps, wp_r[:, o, ts(g, 128)], xpool_r[:, o, :],
                    start=(o == 0), stop=(o == 1),
                )
            nc.vector.tensor_copy(repT[:, g, :], rt_ps)
            for h in range(2):
                rows = slice(64 * h, 64 * h + 64)
                nc.vector.tensor_scalar_mul(
                    repT_bd[rows, g, ds(64 * h, 64)], repT[rows, g, :], SCALE
                )
        nc.vector.tensor_copy(repT_bd_r, repT_bd)
        for p in range(NPAIR):
            for o in range(2):
                ws_ps = psA.tile([128, 128], F32, name="ws_ps", tag="sm",
                                 bufs=2)
                nc.tensor.matmul(
                    ws_ps, wpT_r[:, p, ds(128 * o, 128)], repT_bd_r[:, p, :],
                    start=True, stop=True,
                )
                nc.vector.tensor_scalar_mul(ws8[:, o, p, :], ws_ps, 64.0)

    for j in range(8):
        # f32 -> bf16 casts, one engine per half
        # NOTE: every consumer here reads ONE o-half per instruction.  A
        # both-halves AP spans the o-gap in x_sb, and the dependency
        # tracker's conservative byte-range then overlaps EVERY chunk's
        # DMA -- serializing the consumer behind the whole ingest.
        nc.scalar.copy(x_bf[:, 0, ts(j, 512)], x_sb[:, 0, ts(j, 512)])
        nc.vector.tensor_copy(x_bf[:, 1, ts(j, 512)], x_sb[:, 1, ts(j, 512)])
        nc.scalar.copy(x8[:, 0, ts(j, 512)], x_sb[:, 0, ts(j, 512)])
        nc.gpsimd.tensor_copy(x8[:, 1, ts(j, 512)], x_sb[:, 1, ts(j, 512)])
        # chunk j covers h rows 8j..8j+8 (one block row);
        # local t = hi*64 + wb*8 + wi -> reduce (hi, wi) per block col wb.
        # The pool -> ws chain gates the whole scores phase: high priority
        # so the scheduler runs each pool as its chunk lands.
        with tc.high_priority():
            for o in range(2):
                nc.vector.reduce_sum(
                    xpool_raw[:, o, ts(j, 8)],
                    x_sb[:, o, ts(j, 512)].rearrange(
                        "p (hi wb wi) -> p wb hi wi", hi=8, wb=8),
                    axis=AX.XY,
                )
        if j == 7:
            with tc.high_priority(offset=200):
                _build_ws()
        for m in range(4 * j, 4 * j + 4):
            pr_full = psS.tile([128, 2, 512], F32, name="s_ps", tag="smm",
                               bufs=2)
            pr_ps = pr_full[:, 0, :]
            for o in range(2):
                nc.tensor.matmul(
                    pr_ps, x_bf[:, o, ts(m, 128)], wp_bf[:, o, :],
                    start=(o == 0), stop=(o == 1),
                )
            if m % 4 == 3:
                nc.vector.tensor_copy(proj_bf[:, m, :], pr_ps)
            else:
                nc.scalar.copy(proj_bf[:, m, :], pr_ps)

    # ---- phase 3: scores -> exp -> P^T transpose -> rep_delta bursts ------
    # Group (p, g) covers tokens [1024g, 1024g+1024) for pair p.  P stays
    # unnormalized; 1/Z folds into rep_delta (rz_bc) and V (rz*step_x).
    # |s| <~ 2: no max-subtract.
    p_tiles = [
        pp.tile([128, T], BF16, name=f"p{p}", tag=f"p{p}") for p in range(NPAIR)
    ]
    pt_tiles = [
        ptp.tile([128, NTT, 128], BF16, name=f"pt{p}", tag=f"pt{p}")
        for p in range(NPAIR)
    ]
    zpart_tiles = [
        stats.tile([128, NG], F32, name="zpart", tag=f"zpart{p}")
        for p in range(NPAIR)
    ]

    def _scores(p, g):
        s_ps = psS.tile([128, 2, 512], F32, name="s_ps", tag="smm", bufs=2)
        for c2 in range(2):
            nc.tensor.matmul(
                s_ps[:, c2, :], ws8[:, :, p, :],
                x8[:, :, ds(1024 * g + 512 * c2, 512)],
                start=True, stop=True,
                perf_mode=mybir.MatmulPerfMode.DoubleRow,
            )
        nc.scalar.activation(
            out=p_tiles[p][:, ts(g, 1024)], in_=s_ps, func=ACTF.Exp,
            bias=0.0, scale=1.0 / 64.0, accum_out=zpart_tiles[p][:, g:g + 1],
        )
        eng = nc.sync if (NG * p + g) % 2 == 0 else nc.scalar
        eng.dma_start_transpose(
            pt_tiles[p][:, 8 * g:8 * g + 8, :], p_tiles[p][:, ts(g, 1024)]
        )

    av1_tiles = [None] * NPAIR

    def _av1_burst(p, g):
        if g == 0:
            av1_tiles[p] = psA.tile([128, 128], F32, name="av1_ps", tag="acc",
                                    bufs=2)
        av1_ps = av1_tiles[p]
        for m in range(8 * g, 8 * g + 8):
            nc.tensor.matmul(
                av1_ps, proj_bf[:, m, ds(128 * p, 128)], pt_tiles[p][:, m, :],
                start=(m == 0), stop=(m == NTT - 1),
            )

    # softmax normalizer chain for pair p (issue after its last exp)
    rz_tiles = [None] * NPAIR
    rzbc_tiles = [None] * NPAIR

    def _rz_chain(p):
        zsum = stats.tile([128, 1], F32, name="zsum", tag=f"zsum{p}")
        nc.vector.reduce_sum(zsum, zpart_tiles[p], axis=AX.X)
        rz = stats.tile([128, 1], F32, name="rz", tag=f"rz{p}")
        nc.vector.reciprocal(rz, zsum)
        rz_tiles[p] = rz
        rzt_ps = psA.tile([1, 128], F32, name="rzt_ps", tag="sm", bufs=2)
        nc.tensor.transpose(rzt_ps, rz, ident_f)
        rzt_sb = b3.tile([1, 128], F32, name="rzt_sb", tag="rzt_sb", bufs=4)
        nc.vector.tensor_copy(rzt_sb, rzt_ps)
        rzbc_ps = psA.tile([128, 128], F32, name="rzbc_ps", tag="sm", bufs=2)
        nc.tensor.matmul(rzbc_ps, ones_row, rzt_sb, start=True, stop=True)
        rz_bc = b3.tile([128, 128], F32, name="rz_bc", tag="rz_bc", bufs=2)
        nc.vector.tensor_copy(rz_bc, rzbc_ps)
        rzbc_tiles[p] = rz_bc

    # ---- stage 2 (per pair, issued behind its rep_delta) ------------------
    rep2Tb_tiles = [None] * NPAIR
    rep2Tbd_tiles = [None] * NPAIR
    r2_tiles = [None] * NPAIR
    p2_tiles = [None] * NPAIR
    rz2_tiles = [None] * NPAIR
    v_tiles = [None] * NPAIR

    def _build_rep2(p):
        # rep2^T = repT + step_rep * rz * rep_delta^T   [128 (dA|dB), 64 q]
        av1_ps, rz_bc = av1_tiles[p], rzbc_tiles[p]
        rep2T = b3.tile([128, NB], F32, name="rep2T", tag="rep2T", bufs=2)
        for h in range(2):
            pr = slice(64 * h, 64 * h + 64)
            blk = ds(64 * h, 64)
            nc.vector.scalar_tensor_tensor(
                rep2T[pr, :], av1_ps[pr, blk], srep_bc[pr, p:p + 1],
                rz_bc[pr, blk], op0=ALU.mult, op1=ALU.mult,
            )
        nc.vector.tensor_add(rep2T, rep2T, repT[:, p, :])
        rep2T_b = b3.tile([128, NB], BF16, name="rep2T_b", tag="rep2T_b", bufs=4)
        nc.vector.tensor_copy(rep2T_b, rep2T)
        rep2Tb_tiles[p] = rep2T_b
        rep2T_bd = b3.tile([128, 128], BF16, name="rep2T_bd", tag="rep2T_bd",
                           bufs=4)
        nc.vector.memset(rep2T_bd, 0.0)
        for h in range(2):
            rows = slice(64 * h, 64 * h + 64)
            nc.vector.tensor_scalar_mul(
                rep2T_bd[rows, ds(64 * h, 64)], rep2T[rows, :], SCALE
            )
        rep2Tbd_tiles[p] = rep2T_bd

    def _stage2a(p):
        # rep2 (q' on partitions): [64 q', 128 (dA|dB)]
        r2_ps = psA.tile([64, 128], BF16, name="r2_ps", tag="sm", bufs=2)
        nc.tensor.transpose(r2_ps, rep2Tb_tiles[p], ident_bf)
        r2_sb = b3.tile([64, 128], BF16, name="r2_sb", tag="r2_sb", bufs=4)
        nc.vector.tensor_copy(r2_sb, r2_ps)
        r2_tiles[p] = r2_sb
        # S2 = (scale*rep2) @ rep2.T per head -> [128 (qA|qB), 64 q'].  Only
        # its exp row-sum (Z2) is consumed; exp(S2) itself is taken from the
        # transposed product below (exp(S2) is symmetric per head block).
        s2_ps = psA.tile([128, NB], F32, name="s2_ps", tag="sm", bufs=2)
        nc.tensor.matmul(s2_ps, rep2Tbd_tiles[p], rep2Tb_tiles[p],
                         start=True, stop=True)
        z2 = stats.tile([128, 1], F32, name="z2", tag=f"z2{p}")
        p2_sb = b3.tile([128, NB], BF16, name="p2_sb", tag="p2_sb", bufs=4)
        nc.scalar.activation(
            out=p2_sb, in_=s2_ps, func=ACTF.Exp,
            bias=0.0, scale=1.0, accum_out=z2,
        )
        # S2^T = rep2 @ (scale*rep2)^T -> [64 q', 128 (qA|qB)]; its exp IS
        # P2^T unnormalized (1/Z2 is folded into the V row scale).
        s2t_ps = psA.tile([64, 128], F32, name="s2t_ps", tag="sm", bufs=2)
        nc.tensor.matmul(s2t_ps, rep2Tb_tiles[p], rep2Tbd_tiles[p],
                         start=True, stop=True)
        p2t_sb = b3.tile([64, 128], BF16, name="p2t_sb", tag="p2t_sb", bufs=4)
        nc.scalar.activation(
            out=p2t_sb, in_=s2t_ps, func=ACTF.Exp, bias=0.0, scale=1.0,
        )
        p2_tiles[p] = p2t_sb
        rz2 = stats.tile([128, 1], F32, name="rz2", tag=f"rz2{p}")
        nc.vector.reciprocal(rz2, z2)
        rz2_tiles[p] = rz2

    def _stage2b(p):
        # xd2^T directly: [128 (dA|dB), 128 (qA|qB)] = rep2^T @ P2unnorm^T;
        # diag quadrants real, cross quadrants garbage (zeroed below).
        xd2t_ps = psA.tile([128, 128], F32, name="xd2t_ps", tag="sm", bufs=2)
        nc.tensor.matmul(xd2t_ps, r2_tiles[p], p2_tiles[p],
                         start=True, stop=True)
        xd2bd = b3.tile([128, 128], BF16, name="xd2bd", tag="xd2bd", bufs=4)
        nc.vector.memset(xd2bd, 0.0)
        for h in range(2):
            rows = slice(64 * h, 64 * h + 64)
            nc.vector.tensor_copy(
                xd2bd[rows, ds(64 * h, 64)], xd2t_ps[rows, ds(64 * h, 64)]
            )
        # V_pair^T[q, c] = sum_d xd2_bd^T[d, q] Wo[128p+d, c], then scale
        # rows (queries) by rz * step_x * rz2 (stage-1 and stage-2 softmax
        # normalizers both fold in here).
        v_ps = psA.tile([128, C], F32, name="v_ps", tag="sm", bufs=2)
        nc.tensor.matmul(v_ps, xd2bd, wo_bf[:, p, :], start=True, stop=True)
        rzsx = stats.tile([128, 1], F32, name="rzsx", tag=f"rzsx{p}")
        nc.vector.tensor_mul(rzsx, rz_tiles[p], sx_bc[:, p:p + 1])
        nc.vector.tensor_mul(rzsx, rzsx, rz2_tiles[p])
        v_sb = vp.tile([128, C], BF16, name=f"v{p}", tag=f"v{p}")
        nc.vector.tensor_scalar_mul(v_sb, v_ps, rzsx)
        v_tiles[p] = v_sb

    # pipeline driver: av1 bursts trail the score groups by 2 slots so the
    # PE stream never head-of-line blocks on an in-flight exp/transpose;
    # each pair's rep2 build (DVE) gets a slot of head start before its
    # stage-2 PE ops enter the stream.
    sched = [(p, g) for p in range(NPAIR) for g in range(NG)]
    nsl = len(sched)
    rep2_q = []
    s2_q = []
    for i in range(nsl + 4):
        if i < nsl:
            p, g = sched[i]
            _scores(p, g)
        if s2_q:
            pq = s2_q.pop(0)
            _stage2a(pq)
            _stage2b(pq)
        if rep2_q:
            pq = rep2_q.pop(0)
            _build_rep2(pq)
            s2_q.append(pq)
        if 2 <= i < nsl + 2:
            p2, g2 = sched[i - 2]
            if g2 == NG - 1:
                _rz_chain(p2)
            _av1_burst(p2, g2)
            if g2 == NG - 1:
                rep2_q.append(p2)

    # ---- fused back-projection + output projection -----------------------
    # out[c, t] = sum_p (V_pair^T)^T @ P_pair + b_out; K accumulates both
    # heads of the pair (query index carries head identity on both sides).
    # ct-major: each ct half accumulates into one contiguous staging tile
    # and ships as a single 2MB store (16KB descriptors; the two halves
    # are disjoint DRAM ranges, so the two rings run in parallel).
    xp.release()
    outp = tc.alloc_tile_pool(name="outp", bufs=1)       # out staging
    out_sb = outp.tile([128, 2, T], F32, name="out_sb")
    for ct in range(2):
        for j in range(8):
            op_full = psS.tile([128, 2, 512], F32, name="s_ps", tag="smm",
                               bufs=2)
            op_ps = op_full[:, 0, :]
            for p in range(NPAIR):
                nc.tensor.matmul(
                    op_ps, v_tiles[p][:, ts(ct, 128)],
                    p_tiles[p][:, ts(j, 512)],
                    start=(p == 0), stop=(p == NPAIR - 1),
                )
            if (2 * j + ct) % 2 == 0:
                nc.vector.tensor_tensor(
                    out_sb[:, ct, ts(j, 512)], op_ps,
                    bo_sb[:, ct:ct + 1].to_broadcast((128, 512)),
                    ALU.add,
                )
            else:
                nc.scalar.activation(
                    out=out_sb[:, ct, ts(j, 512)], in_=op_ps,
                    func=ACTF.Identity, bias=bo_sb[:, ct:ct + 1], scale=1.0,
                )
        eng = nc.sync if ct == 0 else nc.scalar
        eng.dma_start(out_r[:, ct, :], out_sb[:, ct, :])

    psS.release()
    psA.release()
    outp.release()
    pnp.release()
    ptp.release()
    b3.release()
    pp.release()
    vp.release()
    stats.release()
    consts.release()


_CACHE = {}


class _Runner:
    """Builds the Bass module once and keeps a single jitted shard_map
    executable alive, so repeat kernel() calls skip retracing/relowering."""

    def __init__(self):
        import jax
        from jax.sharding import Mesh, PartitionSpec
        from jax.experimental.shard_map import shard_map
        from concourse import bass2jax

        self.jax = jax
        nc = build_module()
        self.nc = nc
        bass2jax.install_neuronx_cc_hook()

        partition_name = (
            nc.partition_id_tensor.name if nc.partition_id_tensor else None
        )
        in_names, out_names, out_avals = [], [], []
        for alloc in nc.m.functions[0].allocations:
            if not isinstance(alloc, mybir.MemoryLocationSet):
                continue
            name = alloc.memorylocations[0].name
            if alloc.kind == "ExternalInput":
                if name != partition_name:
                    in_names.append(name)
            elif alloc.kind == "ExternalOutput":
                out_names.append(name)
                out_avals.append(
                    jax.core.ShapedArray(
                        tuple(alloc.tensor_shape), mybir.dt.np(alloc.dtype)
                    )
                )
        n_params = len(in_names)
        n_outs = len(out_avals)
        all_names = list(in_names) + list(out_names)
        if partition_name is not None:
            all_names.append(partition_name)
        self.in_names = in_names
        self.out_names = out_names
        self.out_avals = out_avals

        def _body(*args):
            operands = list(args)
            if partition_name is not None:
                operands.append(bass2jax.partition_id_tensor())
            outs = bass2jax._bass_exec_p.bind(
                *operands,
                out_avals=tuple(out_avals),
                in_names=tuple(all_names),
                out_names=tuple(out_names),
                lowering_input_output_aliases=(),
                sim_require_finite=True,
                sim_require_nnan=True,
                nc=nc,
            )
            return tuple(outs)

        self.body = _body
        devices = jax.devices()[:B]
        mesh = Mesh(np.asarray(devices), ("core",))
        donate = tuple(range(n_params, n_params + n_outs))
        self.sharded = jax.jit(
            shard_map(
                _body, mesh=mesh,
                in_specs=(PartitionSpec("core"),) * (n_params + n_outs),
                out_specs=(PartitionSpec("core"),) * n_outs,
                check_rep=False,
            ),
            donate_argnums=donate,
            keep_unused=True,
        )

    def run(self, in_maps):
        concat_in = [
            np.concatenate([m[name] for m in in_maps], axis=0)
            for name in self.in_names
        ]
        zeros = [
            np.zeros((B * a.shape[0], *a.shape[1:]), a.dtype) for a in self.out_avals
        ]
        out_arrs = self.sharded(*concat_in, *zeros)
        return [
            {
                name: np.asarray(out_arrs[i]).reshape(B, *self.out_avals[i].shape)[c]
                for i, name in enumerate(self.out_names)
            }
            for c in range(B)
        ]

    def bench(self, in_maps, reps=8, inner=72, base=8):
        """Time device-resident executions (no donation, operands staged once).

        Times jitted chains of `base` and `inner` back-to-back kernel
        executions; returns (per_exec_seconds, base_chain_seconds, results)
        with per_exec = (t_inner - t_base) / (inner - base), which amortizes
        away the per-dispatch round-trip of this axon-tunneled environment.
        """
        import time
        from jax.sharding import Mesh, PartitionSpec, NamedSharding
        from jax.experimental.shard_map import shard_map

        jax = self.jax
        devices = jax.devices()[:B]
        mesh = Mesh(np.asarray(devices), ("core",))
        sharding = NamedSharding(mesh, PartitionSpec("core"))
        n_ops = len(self.in_names) + len(self.out_avals)

        def chain(n):
            def f(*args):
                outs = []
                for _ in range(n):
                    outs.extend(self.body(*args))
                return tuple(outs)
            return f

        concat_in = [
            np.concatenate([m[name] for m in in_maps], axis=0)
            for name in self.in_names
        ]
        zeros = [
            np.zeros((B * a.shape[0], *a.shape[1:]), a.dtype) for a in self.out_avals
        ]
        staged = [jax.device_put(a, sharding) for a in concat_in + zeros]

        # The device is occasionally in a degraded mode where chained
        # executions serialize (~10x): retry the whole measurement with
        # freshly traced executables and keep the best estimate.
        per_exec = float("inf")
        tbase_best = float("inf")
        out1 = None
        for attempt in range(4):
            times = {}
            for n in (base, inner):
                jfn = jax.jit(
                    shard_map(
                        chain(n), mesh=mesh,
                        in_specs=(PartitionSpec("core"),) * n_ops,
                        out_specs=(PartitionSpec("core"),) * (n * len(self.out_avals)),
                        check_rep=False,
                    ),
                    keep_unused=True,
                )
                out = jfn(*staged)
                jax.block_until_ready(out)
                best = float("inf")
                for _ in range(reps):
                    t0 = time.perf_counter()
                    out = jfn(*staged)
                    jax.block_until_ready(out)
                    best = min(best, time.perf_counter() - t0)
                times[n] = best
                if n == base and out1 is None:
                    out1 = out
            est = (times[inner] - times[base]) / (inner - base)
            if est <= 0:
                est = times[inner] / inner  # noise floor: report upper bound
            per_exec = min(per_exec, est)
            tbase_best = min(tbase_best, times[base])
            if per_exec < 1.2e-4:
                break

        results = [
            {
                name: np.asarray(out1[i]).reshape(B, *self.out_avals[i].shape)[c]
                for i, name in enumerate(self.out_names)
            }
            for c in range(B)
        ]
        return per_exec, tbase_best, results


def _get_runner():
    key = CFG["p_mode"]
    if key not in _CACHE:
        _CACHE[key] = _Runner()
    return _CACHE[key]


def _make_in_maps(x, W_proj, step_rep, step_x, W_out, b_out):
    x = np.ascontiguousarray(np.asarray(x, dtype=np.float32))
    shared = {
        "w_proj": np.ascontiguousarray(np.asarray(W_proj, dtype=np.float32)),
        "w_out": np.ascontiguousarray(np.asarray(W_out, dtype=np.float32)),
        "b_out": np.ascontiguousarray(np.asarray(b_out, dtype=np.float32)),
        "s_rep": np.ascontiguousarray(
            np.asarray(step_rep, dtype=np.float32).reshape(HEADS)
        ),
        "s_x": np.ascontiguousarray(
            np.asarray(step_x, dtype=np.float32).reshape(HEADS)
        ),
    }
    return [
        {"x": np.ascontiguousarray(x[b].reshape(C, T)), **shared} for b in range(B)
    ]


def kernel(x, W_proj, step_rep, step_x, W_out, b_out):
    runner = _get_runner()
    results = runner.run(_make_in_maps(x, W_proj, step_rep, step_x, W_out, b_out))
    outs = [np.asarray(results[b]["out"]).reshape(C, 64, 64) for b in range(B)]
    return np.stack(outs, axis=0)
